# revision 8
# baseline (speedup 1.0000x reference)
"""Trainium2 Bass kernel for nn_Encoder_78795470012907.

Encoder layer: per-head Q/K/V projections, scores = QK^T/sqrt(dk),
double softmax (over batch axis, then over key axis), Z = pV, concat
heads, output projection. S=2048, B=4, D=512, H=8, dk=dv=64.

Sharding: head-parallel over 8 cores (core h owns head h) for the
attention; AllToAll re-shards by token for the output projection, so
each core emits a disjoint 1024-token slice of the output (host just
concatenates).

Layout notes (per core):
 - tokens are b-major: tok = b*2048 + s.
 - X is fed pre-transposed AND pre-cast from host as XT [D, NTOK] bf16:
   all projection matmuls run at bf16 rate with FWL weight loads, X DMA
   traffic halves, and no on-chip f32->bf16 casts are needed.
 - projections produce Q^T/K^T [dk, tok] stacked in b-pairs; Q/K biases
   ride the projection matmul as a K=1 ones-row term (keeps ACT free
   for the exps, which are the kernel's hard floor at ~1.2G elem/s).
 - scores are computed transposed ([t, s] tiles); the two batches of a
   b-pair are issued as concurrent row-tiled matmuls (tile_position
   (0,0)/(64,0)) so the K=64 contractions share the PE array.
 - softmax over b: e=exp(s/8) -> D=sum_b e -> r=1/D -> p1=e*r, with
   1/D on the custom-DVE fast reciprocal (fp32-only; dd sums to f32
   directly so only one bf16 downcast remains).
 - softmax over the key axis t rides the Z matmul via a ones-column
   appended to V (row 64 of the Z psum accumulates sum_t exp); at each
   sc boundary that row is inverted on DVE (fast reciprocal) BEFORE the
   AllToAll, so phase C needs no Ln/Exp activations (and no ACT table
   loads) - it just partition-broadcasts the shipped 1/D2 row.
 - phase B is software-pipelined two blocks deep so exp1(g+2) precedes
   exp2(g) in the ACT queue; the AllToAll is split in two halves, the
   first overlapping the second half of the attention loop.
"""

from contextlib import ExitStack

import numpy as np
import ml_dtypes

import concourse.bass as bass
import concourse.tile as tile
from concourse import bacc, mybir
from concourse.bass_utils import run_bass_kernel_spmd

S, B, D = 2048, 4, 512
H, DK, DV = 8, 64, 64
N_CORES = 8
NTOK = S * B          # 8192 tokens, b-major
TOKC = NTOK // N_CORES  # 1024 tokens per core for the output slice
SC = 512              # s-chunk (columns of a scores^T tile)
TC = 128              # t-chunk (partitions of a scores^T tile)
N_SC = S // SC        # 4
N_TC = S // TC        # 16

F32 = mybir.dt.float32
BF16 = mybir.dt.bfloat16
AF = mybir.ActivationFunctionType
BF = ml_dtypes.bfloat16


def build_kernel():
    nc = bacc.Bacc(num_devices=N_CORES)

    xt_d = nc.dram_tensor("xt", [D, NTOK], BF16, kind="ExternalInput")
    wqk_d = nc.dram_tensor("wqk", [D, 128], BF16, kind="ExternalInput")
    bqk_d = nc.dram_tensor("bqk", [1, 128], BF16, kind="ExternalInput")
    wv_d = nc.dram_tensor("wv", [D, DV], BF16, kind="ExternalInput")
    bv_d = nc.dram_tensor("bv", [1, DV], BF16, kind="ExternalInput")
    wo_d = nc.dram_tensor("wo", [D, D], BF16, kind="ExternalInput")
    bo_d = nc.dram_tensor("bo", [1, D], BF16, kind="ExternalInput")
    out_d = nc.dram_tensor("out", [TOKC, D], F32, kind="ExternalOutput")

    with tile.TileContext(nc) as tc, ExitStack() as ctx:
        pp = ctx.enter_context(tc.tile_pool(name="persist", bufs=1))
        dram = ctx.enter_context(tc.tile_pool(name="dram", bufs=1, space="DRAM"))

        # ---- persistent SBUF ----
        # Q^T/K^T in b-pairs: rows 0:64 = batch 2p, rows 64:128 = batch 2p+1
        qt = [pp.tile([128, S], BF16, tag=f"qt{p}", name=f"qt{p}") for p in range(2)]
        kt = [pp.tile([128, S], BF16, tag=f"kt{p}", name=f"kt{p}") for p in range(2)]
        # V-tilde: 64 token-chunks of [128 tok, 65] (col 64 = ones)
        vt = pp.tile([128, 64 * 65], BF16, tag="vt", name="vt")
        # Z^T (unnormalized) + 1/denom row: [65, NTOK]
        zt = pp.tile([65, NTOK], BF16, tag="zt", name="zt")

        # weights (all bf16, loaded directly)
        wqk = [pp.tile([128, 128], BF16, tag=f"wqk{i}", name=f"wqk{i}") for i in range(4)]
        wv = [pp.tile([128, DV], BF16, tag=f"wv{i}", name=f"wv{i}") for i in range(4)]
        wo = [pp.tile([128, D], BF16, tag=f"wo{i}", name=f"wo{i}") for i in range(4)]
        bqk = pp.tile([1, 128], BF16, tag="bqk", name="bqk")
        bv = pp.tile([1, DV], BF16, tag="bv", name="bv")
        bo = pp.tile([1, D], BF16, tag="bo", name="bo")

        for i in range(4):
            nc.sync.dma_start(wqk[i][:], wqk_d[i * 128:(i + 1) * 128, :])
            nc.sync.dma_start(wv[i][:], wv_d[i * 128:(i + 1) * 128, :])
            nc.sync.dma_start(wo[i][:], wo_d[i * 128:(i + 1) * 128, :])
        nc.sync.dma_start(bqk[:], bqk_d[:])
        nc.sync.dma_start(bv[:], bv_d[:])
        nc.sync.dma_start(bo[:], bo_d[:])

        onesf = pp.tile([128, 512], F32, tag="onesf", name="onesf")
        nc.vector.memset(onesf[:], 1.0)
        ones_bf = pp.tile([1, 512], BF16, tag="ones_bf", name="ones_bf")
        nc.vector.tensor_copy(ones_bf[:], onesf[0:1, :])
        # ones column (col 64 of each 65-wide group) of V-tilde
        vt_ones = vt[:].rearrange("p (n c) -> p n c", c=65)[:, :, 64:65]
        nc.vector.tensor_copy(vt_ones, onesf[:, 0:64, None])
        # warm the ACT exp table set before phase A so the first attention
        # block doesn't eat the ~2.7us ACT_TABLE_LOAD.
        warm = pp.tile([1, 1], BF16, tag="warm", name="warm")
        nc.scalar.activation(warm[:], onesf[0:1, 0:1], AF.Exp)

        # ================= Phase A: projections =================
        with (
            tc.tile_pool(name="xtp", bufs=2) as xp,
            tc.tile_pool(name="psA", bufs=2, space="PSUM") as psA,
        ):
            # ssub-outer, b-inner: the first 4 chunks cover (ssub=0) of
            # every batch, so attention's sc=0 march can start early.
            for ssub in range(4):
                # one batched DMA per d-block: [128, 4 (b) x 512] strided
                xtile = [xp.tile([128, 4, 512], BF16, tag=f"xt{i}",
                                 name=f"xtile{i}") for i in range(4)]
                xsrc = xt_d[:].rearrange("d (b ss s) -> d b ss s", b=4, ss=4)
                for i in range(4):
                    nc.sync.dma_start(
                        xtile[i][:],
                        xsrc[i * 128:(i + 1) * 128, :, ssub, :],
                    )
                for b in range(4):
                    pair, row = b // 2, (b % 2) * 64
                    # Q^T | K^T (stacked 64+64) for this token chunk
                    pqk = psA.tile([128, 512], F32, tag="pqk", name="pqk")
                    for i in range(4):
                        nc.tensor.matmul(pqk[:], wqk[i][:], xtile[i][:, b, :],
                                         start=(i == 0), stop=False)
                    # bias as a K=1 ones-row term (keeps ACT free)
                    nc.tensor.matmul(pqk[:], bqk[:], ones_bf[:],
                                     start=False, stop=True)
                    scol = ssub * 512
                    nc.vector.tensor_copy(qt[pair][row:row + 64, scol:scol + 512],
                                          pqk[0:64, :])
                    nc.vector.tensor_copy(kt[pair][row:row + 64, scol:scol + 512],
                                          pqk[64:128, :])
                    # V (natural layout) per 128-token subchunk
                    for sub in range(4):
                        pv = psA.tile([128, DV], F32, tag="pv", name="pv")
                        for i in range(4):
                            nc.tensor.matmul(pv[:], xtile[i][:, b, sub * 128:(sub + 1) * 128],
                                             wv[i][:], start=(i == 0), stop=False)
                        nc.tensor.matmul(pv[:], ones_bf[:, 0:128], bv[:],
                                         start=False, stop=True)
                        tci = (b * 4 + ssub) * 4 + sub  # global token-chunk (b-major)
                        nc.vector.tensor_copy(vt[:, tci * 65:tci * 65 + 64], pv[:])

        # ================= Phase B: attention =================
        with (
            tc.tile_pool(name="wb", bufs=2) as wb,
            tc.tile_pool(name="psB", bufs=1, space="PSUM") as psB,
        ):
            # Software-pipelined over 64 global blocks g = sc*16 + t.
            NB = N_SC * N_TC
            pipe = {}  # g -> p1 tile

            def softmax_b(g):
                """scores(g) -> e(g) -> p1(g) tiles (no exp2 yet)."""
                sc, t = g // N_TC, g % N_TC
                scp = psB.tile([128, 4 * SC], F32, tag="scp", name="scp")
                for pair in range(2):
                    for half in range(2):
                        b = pair * 2 + half
                        row = half * 64
                        nc.tensor.matmul(
                            scp[:, b * SC:(b + 1) * SC],
                            kt[pair][row:row + 64, t * TC:(t + 1) * TC],
                            qt[pair][row:row + 64, sc * SC:(sc + 1) * SC],
                            start=True, stop=True,
                        )
                # e = exp(scores/8) for all 4 b
                e = wb.tile([128, 4 * SC], BF16, tag="e", name="e", bufs=3)
                nc.scalar.activation(e[:], scp[:], AF.Exp, scale=0.125)
                # D = sum_b e ; r = 1/D (custom-DVE fast reciprocal keeps
                # ACT on the single exp table set)
                t01 = wb.tile([128, 2 * SC], BF16, tag="t01", name="t01", bufs=2)
                nc.vector.tensor_add(t01[:], e[:, 0:2 * SC], e[:, 2 * SC:4 * SC])
                ddf = wb.tile([128, SC], F32, tag="ddf", name="ddf", bufs=2)
                nc.vector.tensor_add(ddf[:], t01[:, 0:SC], t01[:, SC:2 * SC])
                rf = wb.tile([128, SC], F32, tag="rf", name="rf", bufs=2)
                nc.vector.reciprocal_approx_fast(rf[:], ddf[:])
                rr = wb.tile([128, SC], BF16, tag="rr", name="rr", bufs=2)
                nc.vector.tensor_copy(rr[:], rf[:])
                # p1 = e * r, one TT with r broadcast along the 4-b free dim
                p1 = wb.tile([128, 4 * SC], BF16, tag="p1", name="p1", bufs=3)
                nc.vector.tensor_mul(
                    p1[:].rearrange("p (b s) -> p b s", b=4),
                    e[:].rearrange("p (b s) -> p b s", b=4),
                    rr[:, None, :].broadcast_to([128, 4, SC]),
                )
                pipe[g] = p1

            def exp2_and_z(g, zacc):
                """exp2(g) + Z accumulation (ones-col -> sum_t in row 64)."""
                t = g % N_TC
                p1 = pipe.pop(g)
                q = wb.tile([128, 4 * SC], BF16, tag="q", name="q", bufs=3)
                nc.scalar.activation(q[:], p1[:], AF.Exp)
                for b in range(4):
                    tci = b * 16 + t
                    nc.tensor.matmul(
                        zacc[:, b * SC:(b + 1) * SC],
                        vt[:, tci * 65:(tci + 1) * 65],
                        q[:, b * SC:(b + 1) * SC],
                        start=(t == 0), stop=(t == N_TC - 1),
                    )

            a2a_in_h = [dram.tile([N_CORES * 65, 512], BF16, tag=f"a2a_in{q}",
                                  name=f"a2a_in{q}") for q in range(2)]
            a2a_out_h = [dram.tile([N_CORES * 65, 512], BF16, tag=f"a2a_out{q}",
                                   name=f"a2a_out{q}") for q in range(2)]

            def emit_a2a(q):
                # chunk r = my head's Z^T cols for core r's half-q tokens:
                # tok = (r//2)*S + q*1024 + (r%2)*512 ... +512; two batched
                # DMAs (DMA APs cap at 3 dims) move all 8 chunks.
                dst = a2a_in_h[q][:].rearrange("(b sub p) s -> p b sub s",
                                               b=4, sub=2)
                src = zt[:].rearrange("p (b half sub s) -> p b half sub s",
                                      b=4, half=2, sub=2)
                for sub in range(2):
                    nc.sync.dma_start(dst[:, :, sub, :], src[:, :, q, sub, :])
                nc.gpsimd.collective_compute(
                    "AllToAll",
                    mybir.AluOpType.bypass,
                    replica_groups=[list(range(N_CORES))],
                    ins=[a2a_in_h[q][:].opt()],
                    outs=[a2a_out_h[q][:].opt()],
                )

            zaccs = {}
            for g in range(NB + 2):
                if g < NB:
                    if g % N_TC == 0:
                        zaccs[g // N_TC] = psB.tile([65, 4 * SC], F32,
                                                    tag="zacc", name="zacc")
                    softmax_b(g)
                if g >= 2:
                    gz = g - 2
                    za = zaccs[gz // N_TC]
                    exp2_and_z(gz, za)
                    if gz % N_TC == N_TC - 1:
                        # evacuate Z^T to bf16; invert the denominator row
                        # on DVE so phase C needs no Ln/Exp table loads.
                        sc_done = gz // N_TC
                        ztv = zt[:].rearrange("p (b s8) -> p b s8", b=4)
                        zdst = ztv[:, :, sc_done * SC:(sc_done + 1) * SC]
                        nc.vector.tensor_copy(
                            zdst[0:64],
                            za[0:64, :].rearrange("p (b s) -> p b s", b=4))
                        # fast-recip's custom uop misreads PSUM sources;
                        # stage the denominator row to SBUF first.
                        rzf = wb.tile([1, 4 * SC], F32, tag="rzf", name="rzf",
                                      bufs=2)
                        nc.vector.tensor_copy(rzf[:], za[64:65, :])
                        rz = wb.tile([1, 4 * SC], F32, tag="rz", name="rz",
                                     bufs=2)
                        nc.vector.reciprocal_approx_fast(rz[:], rzf[:])
                        nc.vector.tensor_copy(
                            zdst[64:65],
                            rz[:].rearrange("p (b s) -> p b s", b=4))
                        if sc_done == 1:
                            emit_a2a(0)  # overlaps remaining attention
                        elif sc_done == 3:
                            emit_a2a(1)

        # ================= Phase C: output projection =================
        with (
            tc.tile_pool(name="wc", bufs=1) as wc,
            tc.tile_pool(name="oc", bufs=2) as oc,
            tc.tile_pool(name="psC", bufs=2, space="PSUM") as psC,
        ):
            HT = 512  # tokens per half
            for q in range(2):
                zc = wc.tile([64, N_CORES * HT], BF16, tag="zc", name="zc", bufs=2)
                src = a2a_out_h[q][:].rearrange("(j p) s -> p j s", j=N_CORES)
                nc.sync.dma_start(zc[:].rearrange("p (j s) -> p j s", j=N_CORES),
                                  src[0:64])
                # the shipped denominator row already holds 1/denom
                # (inverted pre-A2A on DVE); land it on partition 0.
                rden = wc.tile([1, N_CORES * HT], BF16, tag="rden", name="rden",
                               bufs=2)
                nc.sync.dma_start(
                    rden[:].rearrange("p (j s) -> p j s", j=N_CORES), src[64:65])
                rb = wc.tile([64, N_CORES * HT], BF16, tag="rb", name="rb", bufs=2)
                nc.gpsimd.partition_broadcast(rb[:], rden[:])
                # normalized Zc^T in hd-major pairs: tile i = heads 2i, 2i+1
                zcn = [wc.tile([128, HT], BF16, tag=f"zcn{i}", name=f"zcn{i}", bufs=2)
                       for i in range(4)]
                for j in range(N_CORES):
                    nc.vector.tensor_mul(
                        zcn[j // 2][(j % 2) * 64:(j % 2) * 64 + 64, :],
                        zc[0:64, j * HT:(j + 1) * HT],
                        rb[:, j * HT:(j + 1) * HT],
                    )
                for m in range(HT // 128):
                    po = psC.tile([128, D], F32, tag="po", name="po")
                    for i in range(4):
                        nc.tensor.matmul(po[:], zcn[i][:, m * 128:(m + 1) * 128],
                                         wo[i][:], start=(i == 0), stop=False)
                    nc.tensor.matmul(po[:], ones_bf[:, 0:128], bo[:], start=False,
                                     stop=True)
                    ot = oc.tile([128, D], F32, tag="ot", name="ot")
                    nc.vector.tensor_copy(ot[:], po[:])
                    row = q * HT + m * 128
                    nc.sync.dma_start(out_d[row:row + 128, :], ot[:])

    nc.compile()
    return nc


_NC_CACHE = None


def _get_nc():
    global _NC_CACHE
    if _NC_CACHE is None:
        _NC_CACHE = build_kernel()
    return _NC_CACHE


def kernel(X, WQ, bQ, WK, bK, WV, bV, WO, bO, _trace=False, _trace_kwargs=None):
    """Full inputs in, full output out. Shards internally across 8 cores."""
    X = np.asarray(X, dtype=np.float32)
    WQ, bQ = np.asarray(WQ, np.float32), np.asarray(bQ, np.float32)
    WK, bK = np.asarray(WK, np.float32), np.asarray(bK, np.float32)
    WV, bV = np.asarray(WV, np.float32), np.asarray(bV, np.float32)
    WO, bO = np.asarray(WO, np.float32), np.asarray(bO, np.float32)
    # [S,B,D] -> XT [D, NTOK] with b-major tokens (tok = b*S + s), bf16
    xt = np.ascontiguousarray(
        X.transpose(2, 1, 0).reshape(D, NTOK)).astype(BF)
    in_maps = []
    for h in range(N_CORES):
        wqk = np.ascontiguousarray(
            np.concatenate([WQ[h], WK[h]], axis=1)).astype(BF)
        bqk = np.ascontiguousarray(
            np.concatenate([bQ[h], bK[h]])[None, :]).astype(BF)
        in_maps.append({
            "xt": xt,
            "wqk": wqk,
            "bqk": bqk,
            "wv": np.ascontiguousarray(WV[h]).astype(BF),
            "bv": np.ascontiguousarray(bV[h][None, :]).astype(BF),
            "wo": np.ascontiguousarray(WO).astype(BF),
            "bo": np.ascontiguousarray(bO[None, :]).astype(BF),
        })
    nc = _get_nc()
    res = run_bass_kernel_spmd(
        nc, in_maps, core_ids=list(range(N_CORES)),
        trace=_trace, **(_trace_kwargs or {}),
    )
    # core c rows: [0:512] = tokens (c//2)*S + (c%2)*512 .. ; [512:1024] same + 1024
    fullb = np.empty((B, S, D), dtype=np.float32)
    for c in range(N_CORES):
        oc = res.results[c]["out"]
        b, off = c // 2, (c % 2) * 512
        fullb[b, off:off + 512] = oc[0:512]
        fullb[b, 1024 + off:1024 + off + 512] = oc[512:1024]
    full = fullb.transpose(1, 0, 2)
    if _trace:
        return np.ascontiguousarray(full), res
    return np.ascontiguousarray(full)
